# revision 32
# baseline (speedup 1.0000x reference)
"""Trainium2 Bass kernel for nn_OA_Layer (offset-attention layer).

Reference (per batch b, C=256, N=4096, CQK=64):
    xs = x + xyz
    q = k = wqk @ xs + bqk          [64, N]
    v = wv @ xs + bv                [C, N]
    E = q^T q                       [N, N]  (symmetric, since q == k)
    attn = softmax(E, rows) ; attn /= (1e-9 + attn.sum(rows))
    x_r = v @ attn
    t = wt @ (xs - x_r) + bt ; t = BN(t) ; x_r = leaky_relu(t, 0.2)
    out = xs + x_r

Sharding: data-parallel over batch B=8 across 8 cores (1 batch/core).

Math restructuring (exact up to fp rounding):
  - pass 1: rowsum[n] = sum_m exp(E[n,m] - diag[n]) via fused Exp+accum
    (diag[n] = ||q_n||^2 guards overflow; softmax is shift-invariant)
  - pass 1 stores the exp'd tiles (bf16) to a DRAM scratch, so pass 2
    needs neither the E matmuls nor a second exp pass: it is a pure bf16
    matmul stream (xr += vT_scaled @ A ; colsum += invrs^T @ A).
  - x_r = (v @ a2) * invcs[m] ; bv folded into bt' = bt - wt @ bv on host.
  - BN+bias folded to t*g + bp_eff on host.
"""

import numpy as np

import concourse.bass as bass
import concourse.tile as tile
from concourse import bacc, mybir
from concourse._compat import with_exitstack

# NOTE: walrus's --enable-ldw-opt was tried and rejected: bass emits
# standalone InstLdweights (for semaphore waits), which that pass refuses
# ("InstLdweights is not compatible with LDW optimization").

F32 = mybir.dt.float32
F32R = mybir.dt.float32r
BF16 = mybir.dt.bfloat16

C = 256
CQK = 64
P = 128
BN_EPS = 1e-5


def build_kernel(N=4096, debug=False):
    """Builds the per-core bass program. Returns nc."""
    nc = bacc.Bacc("TRN2", target_bir_lowering=False, debug=debug,
                   num_devices=8)

    x_d = nc.declare_dram_parameter("x", [C, N], F32, isOutput=False)
    xyz_d = nc.declare_dram_parameter("xyz", [C, N], F32, isOutput=False)
    wqkT_d = nc.declare_dram_parameter("wqkT", [C, CQK], F32, isOutput=False)
    wvT_d = nc.declare_dram_parameter("wvT", [C, C], F32, isOutput=False)
    wtT_d = nc.declare_dram_parameter("wtT", [C, C], F32, isOutput=False)
    bqk_d = nc.declare_dram_parameter("bqk", [CQK, 1], F32, isOutput=False)
    g_d = nc.declare_dram_parameter("g", [C, 1], F32, isOutput=False)
    bp_d = nc.declare_dram_parameter("bp", [C, 1], F32, isOutput=False)
    out_d = nc.declare_dram_parameter("out", [C, N], F32, isOutput=True)

    with tile.TileContext(nc) as tc:
        _emit(nc, tc, N,
              x_d, xyz_d, wqkT_d, wvT_d, wtT_d, bqk_d, g_d, bp_d, out_d)
    nc.compile()
    return nc


@with_exitstack
def _emit(ctx, nc, tc, N,
          x_d, xyz_d, wqkT_d, wvT_d, wtT_d, bqk_d, g_d, bp_d, out_d):
    NB = N // P          # n row-blocks of 128
    MC = N // 512        # m chunks of 512
    ek = ctx.enter_context

    consts = ek(tc.tile_pool(name="consts", bufs=1))
    big = ek(tc.tile_pool(name="big", bufs=1))
    stats = ek(tc.tile_pool(name="stats", bufs=1))

    # ---- constant / resident tensors ----
    wqkT_f = consts.tile([P, 2 * CQK], F32)       # [p, (khalf, o)]
    nc.sync.dma_start(wqkT_f[:].rearrange("p (t m) -> p t m", t=2),
                      wqkT_d[:].rearrange("(t p) m -> p t m", p=P))
    wvT_f = consts.tile([P, 2 * C], F32)
    nc.sync.dma_start(wvT_f[:].rearrange("p (t m) -> p t m", t=2),
                      wvT_d[:].rearrange("(t p) m -> p t m", p=P))
    wtT = consts.tile([P, 2 * C], F32)
    nc.sync.dma_start(wtT[:].rearrange("p (t m) -> p t m", t=2),
                      wtT_d[:].rearrange("(t p) m -> p t m", p=P))
    wqkT = consts.tile([P, 2 * CQK], F32R)
    nc.vector.tensor_copy(wqkT[:], wqkT_f[:])
    wvT = consts.tile([P, 2 * C], F32R)
    nc.vector.tensor_copy(wvT[:], wvT_f[:])
    bqk = consts.tile([CQK, 1], F32)
    nc.sync.dma_start(bqk[:], bqk_d[:])
    g_t = consts.tile([P, 2], F32)
    bp_t = consts.tile([P, 2], F32)
    for h in range(2):
        nc.sync.dma_start(g_t[:, h:h + 1], g_d[h * P:(h + 1) * P, :])
        nc.sync.dma_start(bp_t[:, h:h + 1], bp_d[h * P:(h + 1) * P, :])
    ones64_f = consts.tile([CQK, 1], F32)
    nc.vector.memset(ones64_f[:], 1.0)
    ones64 = consts.tile([CQK, 1], F32R)
    nc.vector.tensor_copy(ones64[:], ones64_f[:])
    wtT_b = consts.tile([P, 2 * C], BF16)
    nc.vector.tensor_copy(wtT_b[:], wtT[:])

    # xs = x + xyz, layout [128, 2*N] (c-half h at cols [h*N, (h+1)*N)).
    # Stored as f32r so the q/v matmuls run at full PE rate.
    xs = big.tile([P, 2 * N], F32R)
    zpool = ek(tc.tile_pool(name="zpool", bufs=2))
    ZW = 2048
    for z0 in range(0, N, ZW):
        for h in range(2):
            xin = zpool.tile([P, ZW], F32, tag="xin")
            nc.sync.dma_start(xin[:], x_d[h * P:(h + 1) * P, z0:z0 + ZW])
            zin = zpool.tile([P, ZW], F32, tag="zin")
            nc.sync.dma_start(zin[:], xyz_d[h * P:(h + 1) * P, z0:z0 + ZW])
            nc.vector.tensor_add(xs[:, h * N + z0: h * N + z0 + ZW],
                                 xin[:], zin[:])

    # q2: q duplicated on partition halves 0-63 / 64-127 (for PE row-packing)
    q2 = big.tile([P, N], F32R)
    # v^T tile i at cols [i*C, (i+1)*C); bf16 halves the xr LDWEIGHTS cost
    vT = big.tile([P, NB * C], BF16)

    # ---- q = wqk @ xs + bqk ; v^T = xs^T @ wv^T ----
    with tc.tile_pool(name="qvps", bufs=2, space=bass.MemorySpace.PSUM) as qvps:
        for j in range(MC):
            q_ps = qvps.tile([CQK, 512], F32, tag="q_ps")
            for k in range(2):
                nc.tensor.matmul(q_ps[:], wqkT[:, k * CQK:(k + 1) * CQK],
                                 xs[:, k * N + j * 512: k * N + j * 512 + 512],
                                 start=(k == 0), stop=(k == 1))
            nc.vector.tensor_scalar_add(q2[0:CQK, j * 512:(j + 1) * 512],
                                        q_ps[:], bqk[:])
        nc.sync.dma_start(q2[CQK:P, :], q2[0:CQK, :])
        # diag[n] = ||q_n||^2 ; negdiag used as per-row exp shift (overflow fix)
        diag_row = stats.tile([1, N], F32)
        sqp = tc.tile_pool(name="sqp", bufs=2)
        with sqp as sqpool:
            for j in range(MC):
                sq = sqpool.tile([CQK, 512], F32R, tag="sq")
                qs = q2[0:CQK, j * 512:(j + 1) * 512].bitcast(F32)
                nc.vector.tensor_mul(sq[:], qs, qs)
                dg_ps = qvps.tile([1, 512], F32, tag="dg_ps")
                nc.tensor.matmul(dg_ps[:], ones64[:], sq[:],
                                 start=True, stop=True)
                nc.vector.tensor_scalar_mul(diag_row[:, j * 512:(j + 1) * 512],
                                            dg_ps[:], -1.0)
        negdiag = stats.tile([P, NB], F32)
        for i in range(NB):
            nc.sync.dma_start(negdiag[:, i:i + 1],
                              diag_row[0:1, i * P:(i + 1) * P])
        for i in range(NB):
            v_ps = qvps.tile([P, C], F32, tag="v_ps")
            for k in range(2):
                nc.tensor.matmul(v_ps[:],
                                 xs[:, k * N + i * P: k * N + i * P + P],
                                 wvT[:, k * C:(k + 1) * C],
                                 start=(k == 0), stop=(k == 1))
            nc.vector.tensor_copy(vT[:, i * C:(i + 1) * C], v_ps[:])

    # ---- pass 1: rowsums of exp(E - diag); A tiles stored to DRAM (bf16) ----
    # The exp'd tiles are written to a DRAM scratch so pass 2 needs neither
    # the E matmuls nor a second exp pass — it's a pure bf16 matmul stream.
    # Layout is i-major: tile (i, j) at cols (i*MC + j)*512. Pass-1 strips
    # write [P, 2048] contiguously (4KB per-partition descriptors); pass-2
    # reads [P, 1024] chunk-pairs contiguously (2KB descriptors).
    adram = ek(tc.tile_pool(name="adram", bufs=1, space="DRAM"))
    a_dram = adram.tile([P, MC * NB * 512], BF16)
    SW = min(2048, N)              # strip width
    SPB = N // SW                  # strips per block
    CPS = SW // 512                # 512-chunks per strip
    rs_acc = stats.tile([P, SPB * NB], F32)
    with (
        tc.tile_pool(name="p1ps", bufs=2, space=bass.MemorySpace.PSUM) as p1ps,
        tc.tile_pool(name="p1sc", bufs=3) as p1sc,
    ):
        for i in range(NB):
            for s in range(SPB):
                estrip = p1ps.tile([P, SW], F32, tag="estrip")
                for jj in range(CPS):
                    m0 = s * SW + jj * 512
                    qrow = (CQK if jj % 2 == 1 else 0)
                    nc.tensor.matmul(
                        estrip[:, jj * 512:(jj + 1) * 512],
                        q2[qrow:qrow + CQK, i * P:(i + 1) * P],
                        q2[qrow:qrow + CQK, m0:m0 + 512],
                        start=True, stop=True)
                sink = p1sc.tile([P, SW], BF16, tag="sink")
                nc.scalar.activation(
                    sink[:], estrip[:], mybir.ActivationFunctionType.Exp,
                    bias=negdiag[:, i:i + 1],
                    accum_out=rs_acc[:, i * SPB + s: i * SPB + s + 1])
                nc.sync.dma_start(
                    a_dram[:, (i * MC + s * CPS) * 512:
                           (i * MC + (s + 1) * CPS) * 512],
                    sink[:])

    # invrs = 1/rowsum; folded into vT columns (and the colsum stationary)
    rs_sum = stats.tile([P, NB], F32)
    if SPB == 2:
        nc.vector.tensor_add(rs_sum[:], rs_acc[:, 0:2 * NB:2],
                             rs_acc[:, 1:2 * NB:2])
    else:
        nc.vector.tensor_copy(rs_sum[:], rs_acc[:])
    invrs_f = stats.tile([P, NB], F32)
    nc.vector.reciprocal(invrs_f[:], rs_sum[:])
    invrs_bf = stats.tile([P, NB], BF16)
    nc.vector.tensor_copy(invrs_bf[:], invrs_f[:])
    for i in range(NB):
        nc.vector.tensor_scalar_mul(vT[:, i * C:(i + 1) * C],
                                    vT[:, i * C:(i + 1) * C],
                                    invrs_f[:, i:i + 1])

    # ---- pass 2: a2 = attn tile, xr += vT@a2, colsum += ones@a2 ----
    # One 512-col chunk per step; E matmul of step i+1 is emitted before
    # xr/cs of step i (software pipelining) so the in-order PE queue always
    # has ready work while act(i) completes. a2/vT/ones are bf16: same PE
    # rate, half the LDWEIGHTS traffic.
    with (
        tc.tile_pool(name="xrps", bufs=1, space=bass.MemorySpace.PSUM) as xrps,
        tc.tile_pool(name="csps", bufs=1, space=bass.MemorySpace.PSUM) as csps,
        tc.tile_pool(name="tpps", bufs=1, space=bass.MemorySpace.PSUM) as tpps,
        tc.tile_pool(name="a2p", bufs=3) as a2p,
        tc.tile_pool(name="tails", bufs=1) as tails,
    ):
        # Process chunk PAIRS: each step reads one contiguous [P, 1024] A
        # slab (tiles (i, j) and (i, j+1)), prefetched one step ahead.
        NPAIR = MC // 2

        def fetch(jp, i):
            blk = a2p.tile([P, 1024], BF16, tag="a2", name=f"ab{jp}_{i}")
            base = (i * MC + jp * 2) * 512
            nc.sync.dma_start(blk[:], a_dram[:, base: base + 1024])
            return blk

        blk = fetch(0, 0)
        cur = None
        for jp in range(NPAIR):
            m0 = jp * 1024
            colsum_ps = [csps.tile([1, 512], F32, tag=f"cs{c}",
                                   name=f"cs{c}_{jp}") for c in range(2)]
            xr = [[xrps.tile([P, 512], F32, tag=f"xr{h}{c}",
                             name=f"xr{h}{c}_{jp}") for c in range(2)]
                  for h in range(2)]
            for i in range(NB):
                first, last = (i == 0), (i == NB - 1)
                cur = blk
                if i + 1 < NB:
                    blk = fetch(jp, i + 1)
                elif jp + 1 < NPAIR:
                    blk = fetch(jp + 1, 0)
                for h in range(2):
                    for c in range(2):
                        nc.tensor.matmul(
                            xr[h][c][:],
                            vT[:, i * C + h * P: i * C + h * P + P],
                            cur[:, c * 512:(c + 1) * 512],
                            start=first, stop=last)
                for c in range(2):
                    nc.tensor.matmul(colsum_ps[c][:], invrs_bf[:, i:i + 1],
                                     cur[:, c * 512:(c + 1) * 512],
                                     start=first, stop=last)

            # tail for this chunk pair. xr is copied to SBUF immediately so
            # the xr psum banks free up for the next pair while the rest of
            # the tail (reciprocal chain, t conv, BN, lrelu) runs. invcs
            # broadcast runs on the otherwise-idle GpSimd engine.
            xr_sb = [[None, None], [None, None]]
            for k in range(2):
                for c in range(2):
                    xc = tails.tile([P, 512], F32, tag=f"xc{k}{c}")
                    nc.vector.tensor_copy(xc[:], xr[k][c][:])
                    xr_sb[k][c] = xc
            cs_eps = tails.tile([1, 1024], F32, tag="cse")
            for c in range(2):
                nc.vector.tensor_scalar_add(
                    cs_eps[:, c * 512:(c + 1) * 512], colsum_ps[c][:], 1e-9)
            invr = tails.tile([1, 1024], F32, tag="invr")
            nc.vector.reciprocal(invr[:], cs_eps[:])
            invcs_bc = tails.tile([P, 1024], F32, tag="ib")
            nc.gpsimd.partition_broadcast(invcs_bc[:], invr[:])
            for c in range(2):
                mc0 = m0 + c * 512
                ys = []
                for k in range(2):
                    tmp = tails.tile([P, 512], F32, tag=f"tmp{k}{c}")
                    nc.vector.tensor_mul(tmp[:], xr_sb[k][c][:],
                                         invcs_bc[:, c * 512:(c + 1) * 512])
                    y_h = tails.tile([P, 512], BF16, tag=f"y{k}{c}")
                    nc.vector.tensor_sub(
                        y_h[:],
                        xs[:, k * N + mc0: k * N + mc0 + 512].bitcast(F32),
                        tmp[:])
                    ys.append(y_h)
                for ho in range(2):
                    t_ps = tpps.tile([P, 512], F32, tag="tp",
                                     name=f"tps{ho}{c}_{jp}")
                    for k in range(2):
                        nc.tensor.matmul(
                            t_ps[:],
                            wtT_b[:, k * C + ho * P: k * C + ho * P + P],
                            ys[k][:], start=(k == 0), stop=(k == 1))
                    bn = tails.tile([P, 512], F32, tag=f"bn{ho}{c}")
                    nc.vector.tensor_scalar(bn[:], t_ps[:], g_t[:, ho:ho + 1],
                                            bp_t[:, ho:ho + 1],
                                            mybir.AluOpType.mult,
                                            mybir.AluOpType.add)
                    lr = tails.tile([P, 512], F32, tag=f"lr{ho}{c}")
                    nc.vector.scalar_tensor_tensor(lr[:], bn[:], 0.2, bn[:],
                                                   mybir.AluOpType.mult,
                                                   mybir.AluOpType.max)
                    o_t = tails.tile([P, 512], F32, tag=f"o{ho}{c}")
                    nc.vector.tensor_add(
                        o_t[:], lr[:],
                        xs[:, ho * N + mc0: ho * N + mc0 + 512].bitcast(F32))
                    nc.sync.dma_start(
                        out_d[ho * P:(ho + 1) * P, mc0:mc0 + 512], o_t[:])


# ---------------------------------------------------------------------------
# host-side wrapper
# ---------------------------------------------------------------------------
_NC_CACHE = {}


def _get_nc(N=4096):
    if N not in _NC_CACHE:
        _NC_CACHE[N] = build_kernel(N=N)
    return _NC_CACHE[N]


def host_prep(wqk, bqk, wv, bv, wt, bt, bn_gamma, bn_beta, bn_mean, bn_var):
    wqk = np.asarray(wqk, np.float32)
    wv = np.asarray(wv, np.float32)
    wt = np.asarray(wt, np.float32)
    g = (np.asarray(bn_gamma, np.float32)
         / np.sqrt(np.asarray(bn_var, np.float32) + BN_EPS))
    bp = np.asarray(bn_beta, np.float32) - np.asarray(bn_mean, np.float32) * g
    btp = np.asarray(bt, np.float32) - wt @ np.asarray(bv, np.float32)
    bp_eff = btp * g + bp
    return {
        "wqkT": np.ascontiguousarray(wqk.T),
        "wvT": np.ascontiguousarray(wv.T),
        "wtT": np.ascontiguousarray(wt.T),
        "bqk": np.asarray(bqk, np.float32).reshape(CQK, 1),
        "g": g.reshape(C, 1),
        "bp": bp_eff.reshape(C, 1),
    }


def kernel(x, xyz, wqk, bqk, wv, bv, wt, bt, bn_gamma, bn_beta, bn_mean,
           bn_var, _profile=False):
    from concourse.bass_utils import run_bass_kernel_spmd

    x = np.asarray(x, np.float32)
    xyz = np.asarray(xyz, np.float32)
    B, Cc, N = x.shape
    assert Cc == C and B == 8
    nc = _get_nc(N)
    wmap = host_prep(wqk, bqk, wv, bv, wt, bt, bn_gamma, bn_beta, bn_mean,
                     bn_var)
    in_maps = [
        {"x": np.ascontiguousarray(x[b]),
         "xyz": np.ascontiguousarray(xyz[b]), **wmap}
        for b in range(B)
    ]
    res = run_bass_kernel_spmd(nc, in_maps, list(range(8)), trace=_profile)
    out = np.stack([res.results[b]["out"] for b in range(B)], axis=0)
    if _profile:
        return out, res
    return out


# revision 34
# speedup vs baseline: 1.0799x; 1.0799x over previous
"""Trainium2 Bass kernel for nn_OA_Layer (offset-attention layer).

Reference (per batch b, C=256, N=4096, CQK=64):
    xs = x + xyz
    q = k = wqk @ xs + bqk          [64, N]
    v = wv @ xs + bv                [C, N]
    E = q^T q                       [N, N]  (symmetric, since q == k)
    attn = softmax(E, rows) ; attn /= (1e-9 + attn.sum(rows))
    x_r = v @ attn
    t = wt @ (xs - x_r) + bt ; t = BN(t) ; x_r = leaky_relu(t, 0.2)
    out = xs + x_r

Sharding: data-parallel over batch B=8 across 8 cores (1 batch/core).

Math restructuring (exact up to fp rounding):
  - pass 1: rowsum[n] = sum_m exp(E[n,m] - diag[n]) via fused Exp+accum
    (diag[n] = ||q_n||^2 guards overflow; softmax is shift-invariant)
  - pass 1 stores the exp'd tiles (bf16) to a DRAM scratch, so pass 2
    needs neither the E matmuls nor a second exp pass: it is a pure bf16
    matmul stream (xr += vT_scaled @ A ; colsum += invrs^T @ A).
  - x_r = (v @ a2) * invcs[m] ; bv folded into bt' = bt - wt @ bv on host.
  - BN+bias folded to t*g + bp_eff on host.
"""

import numpy as np

import concourse.bass as bass
import concourse.tile as tile
from concourse import bacc, mybir
from concourse._compat import with_exitstack

# NOTE: walrus's --enable-ldw-opt was tried and rejected: bass emits
# standalone InstLdweights (for semaphore waits), which that pass refuses
# ("InstLdweights is not compatible with LDW optimization").

F32 = mybir.dt.float32
F32R = mybir.dt.float32r
BF16 = mybir.dt.bfloat16

C = 256
CQK = 64
P = 128
BN_EPS = 1e-5


def build_kernel(N=4096, debug=False):
    """Builds the per-core bass program. Returns nc."""
    nc = bacc.Bacc("TRN2", target_bir_lowering=False, debug=debug,
                   num_devices=8)

    x_d = nc.declare_dram_parameter("x", [C, N], F32, isOutput=False)
    xyz_d = nc.declare_dram_parameter("xyz", [C, N], F32, isOutput=False)
    wqkT_d = nc.declare_dram_parameter("wqkT", [C, CQK], F32, isOutput=False)
    wvT_d = nc.declare_dram_parameter("wvT", [C, C], F32, isOutput=False)
    wtT_d = nc.declare_dram_parameter("wtT", [C, C], F32, isOutput=False)
    bqk_d = nc.declare_dram_parameter("bqk", [CQK, 1], F32, isOutput=False)
    g_d = nc.declare_dram_parameter("g", [C, 1], F32, isOutput=False)
    bp_d = nc.declare_dram_parameter("bp", [C, 1], F32, isOutput=False)
    out_d = nc.declare_dram_parameter("out", [C, N], F32, isOutput=True)

    with tile.TileContext(nc) as tc:
        _emit(nc, tc, N,
              x_d, xyz_d, wqkT_d, wvT_d, wtT_d, bqk_d, g_d, bp_d, out_d)
    nc.compile()
    return nc


@with_exitstack
def _emit(ctx, nc, tc, N,
          x_d, xyz_d, wqkT_d, wvT_d, wtT_d, bqk_d, g_d, bp_d, out_d):
    NB = N // P          # n row-blocks of 128
    MC = N // 512        # m chunks of 512
    ek = ctx.enter_context

    consts = ek(tc.tile_pool(name="consts", bufs=1))
    big = ek(tc.tile_pool(name="big", bufs=1))
    stats = ek(tc.tile_pool(name="stats", bufs=1))

    # ---- constant / resident tensors ----
    wqkT_f = consts.tile([P, 2 * CQK], F32)       # [p, (khalf, o)]
    nc.sync.dma_start(wqkT_f[:].rearrange("p (t m) -> p t m", t=2),
                      wqkT_d[:].rearrange("(t p) m -> p t m", p=P))
    wvT_f = consts.tile([P, 2 * C], F32)
    nc.sync.dma_start(wvT_f[:].rearrange("p (t m) -> p t m", t=2),
                      wvT_d[:].rearrange("(t p) m -> p t m", p=P))
    wtT = consts.tile([P, 2 * C], F32)
    nc.sync.dma_start(wtT[:].rearrange("p (t m) -> p t m", t=2),
                      wtT_d[:].rearrange("(t p) m -> p t m", p=P))
    wqkT = consts.tile([P, 2 * CQK], F32R)
    nc.vector.tensor_copy(wqkT[:], wqkT_f[:])
    wvT = consts.tile([P, 2 * C], F32R)
    nc.vector.tensor_copy(wvT[:], wvT_f[:])
    bqk = consts.tile([CQK, 1], F32)
    nc.sync.dma_start(bqk[:], bqk_d[:])
    g_t = consts.tile([P, 2], F32)
    bp_t = consts.tile([P, 2], F32)
    for h in range(2):
        nc.sync.dma_start(g_t[:, h:h + 1], g_d[h * P:(h + 1) * P, :])
        nc.sync.dma_start(bp_t[:, h:h + 1], bp_d[h * P:(h + 1) * P, :])
    ones64_f = consts.tile([CQK, 1], F32)
    nc.vector.memset(ones64_f[:], 1.0)
    ones64 = consts.tile([CQK, 1], F32R)
    nc.vector.tensor_copy(ones64[:], ones64_f[:])
    wtT_r = consts.tile([P, 2 * C], F32R)
    nc.vector.tensor_copy(wtT_r[:], wtT[:])

    # xs = x + xyz, layout [128, 2*N] (c-half h at cols [h*N, (h+1)*N)).
    # Stored as f32r so the q/v matmuls run at full PE rate.
    xs = big.tile([P, 2 * N], F32R)
    zpool = ek(tc.tile_pool(name="zpool", bufs=2))
    ZW = 2048
    for z0 in range(0, N, ZW):
        for h in range(2):
            xin = zpool.tile([P, ZW], F32, tag="xin")
            nc.sync.dma_start(xin[:], x_d[h * P:(h + 1) * P, z0:z0 + ZW])
            zin = zpool.tile([P, ZW], F32, tag="zin")
            nc.sync.dma_start(zin[:], xyz_d[h * P:(h + 1) * P, z0:z0 + ZW])
            nc.vector.tensor_add(xs[:, h * N + z0: h * N + z0 + ZW],
                                 xin[:], zin[:])

    # q2: q duplicated on partition halves 0-63 / 64-127 (for PE row-packing)
    q2 = big.tile([P, N], F32R)
    # v^T tile i at cols [i*C, (i+1)*C); bf16 halves the xr LDWEIGHTS cost
    vT = big.tile([P, NB * C], BF16)

    # ---- q = wqk @ xs + bqk ; v^T = xs^T @ wv^T ----
    with tc.tile_pool(name="qvps", bufs=2, space=bass.MemorySpace.PSUM) as qvps:
        for j in range(MC):
            q_ps = qvps.tile([CQK, 512], F32, tag="q_ps")
            for k in range(2):
                nc.tensor.matmul(q_ps[:], wqkT[:, k * CQK:(k + 1) * CQK],
                                 xs[:, k * N + j * 512: k * N + j * 512 + 512],
                                 start=(k == 0), stop=(k == 1))
            nc.vector.tensor_scalar_add(q2[0:CQK, j * 512:(j + 1) * 512],
                                        q_ps[:], bqk[:])
        nc.sync.dma_start(q2[CQK:P, :], q2[0:CQK, :])
        # diag[n] = ||q_n||^2 ; negdiag used as per-row exp shift (overflow fix)
        diag_row = stats.tile([1, N], F32)
        sqp = tc.tile_pool(name="sqp", bufs=2)
        with sqp as sqpool:
            for j in range(MC):
                sq = sqpool.tile([CQK, 512], F32R, tag="sq")
                qs = q2[0:CQK, j * 512:(j + 1) * 512].bitcast(F32)
                nc.vector.tensor_mul(sq[:], qs, qs)
                dg_ps = qvps.tile([1, 512], F32, tag="dg_ps")
                nc.tensor.matmul(dg_ps[:], ones64[:], sq[:],
                                 start=True, stop=True)
                nc.vector.tensor_scalar_mul(diag_row[:, j * 512:(j + 1) * 512],
                                            dg_ps[:], -1.0)
        negdiag = stats.tile([P, NB], F32)
        for i in range(NB):
            nc.sync.dma_start(negdiag[:, i:i + 1],
                              diag_row[0:1, i * P:(i + 1) * P])
        for i in range(NB):
            v_ps = qvps.tile([P, C], F32, tag="v_ps")
            for k in range(2):
                nc.tensor.matmul(v_ps[:],
                                 xs[:, k * N + i * P: k * N + i * P + P],
                                 wvT[:, k * C:(k + 1) * C],
                                 start=(k == 0), stop=(k == 1))
            nc.vector.tensor_copy(vT[:, i * C:(i + 1) * C], v_ps[:])

    # ---- pass 1: rowsums of exp(E - diag); A tiles stored to DRAM (bf16) ----
    # The exp'd tiles are written to a DRAM scratch so pass 2 needs neither
    # the E matmuls nor a second exp pass — it's a pure bf16 matmul stream.
    # Layout is j-major: tile (i, j) at cols (j*NB + i)*512, so pass 2 can
    # fetch 4 consecutive i-tiles of one chunk j in a single 512KB DMA.
    adram = ek(tc.tile_pool(name="adram", bufs=1, space="DRAM"))
    a_dram = adram.tile([P, MC * NB * 512], BF16)
    SW = min(2048, N)              # strip width
    SPB = N // SW                  # strips per block
    CPS = SW // 512                # 512-chunks per strip
    rs_acc = stats.tile([P, SPB * NB], F32)
    a_dram_v = a_dram[:].rearrange("p (j n f) -> p j n f", j=MC, f=512)
    with (
        tc.tile_pool(name="p1ps", bufs=2, space=bass.MemorySpace.PSUM) as p1ps,
        tc.tile_pool(name="p1sc", bufs=2) as p1sc,
    ):
        for i in range(NB):
            for s in range(SPB):
                estrip = p1ps.tile([P, SW], F32, tag="estrip")
                for jj in range(CPS):
                    m0 = s * SW + jj * 512
                    qrow = (CQK if jj % 2 == 1 else 0)
                    nc.tensor.matmul(
                        estrip[:, jj * 512:(jj + 1) * 512],
                        q2[qrow:qrow + CQK, i * P:(i + 1) * P],
                        q2[qrow:qrow + CQK, m0:m0 + 512],
                        start=True, stop=True)
                sink = p1sc.tile([P, SW], BF16, tag="sink")
                nc.scalar.activation(
                    sink[:], estrip[:], mybir.ActivationFunctionType.Exp,
                    bias=negdiag[:, i:i + 1],
                    accum_out=rs_acc[:, i * SPB + s: i * SPB + s + 1])
                nc.sync.dma_start(
                    a_dram_v[:, s * CPS:(s + 1) * CPS, i, :],
                    sink[:].rearrange("p (j f) -> p j f", f=512))

    # invrs = 1/rowsum; folded into vT columns (and the colsum stationary)
    rs_sum = stats.tile([P, NB], F32)
    if SPB == 2:
        nc.vector.tensor_add(rs_sum[:], rs_acc[:, 0:2 * NB:2],
                             rs_acc[:, 1:2 * NB:2])
    else:
        nc.vector.tensor_copy(rs_sum[:], rs_acc[:])
    invrs_f = stats.tile([P, NB], F32)
    nc.vector.reciprocal(invrs_f[:], rs_sum[:])
    invrs_bf = stats.tile([P, NB], BF16)
    nc.vector.tensor_copy(invrs_bf[:], invrs_f[:])
    for i in range(NB):
        nc.vector.tensor_scalar_mul(vT[:, i * C:(i + 1) * C],
                                    vT[:, i * C:(i + 1) * C],
                                    invrs_f[:, i:i + 1])

    # ---- pass 2: a2 = attn tile, xr += vT@a2, colsum += ones@a2 ----
    # One 512-col chunk per step; E matmul of step i+1 is emitted before
    # xr/cs of step i (software pipelining) so the in-order PE queue always
    # has ready work while act(i) completes. a2/vT/ones are bf16: same PE
    # rate, half the LDWEIGHTS traffic.
    with (
        tc.tile_pool(name="xrps", bufs=1, space=bass.MemorySpace.PSUM) as xrps,
        tc.tile_pool(name="csps", bufs=2, space=bass.MemorySpace.PSUM) as csps,
        tc.tile_pool(name="tpps", bufs=2, space=bass.MemorySpace.PSUM) as tpps,
        tc.tile_pool(name="a2p", bufs=3) as a2p,
        tc.tile_pool(name="tails", bufs=2) as tails,
    ):
        # A tiles come back from DRAM in 4-tile groups (one 512KB DMA each),
        # prefetched one group ahead of the matmul stream.
        def fetch(j, i):
            blk = a2p.tile([P, 4 * 512], BF16, tag="a2", name=f"ab{j}_{i}")
            base = (j * NB + i) * 512
            nc.sync.dma_start(blk[:], a_dram[:, base: base + 4 * 512])
            return blk

        blk = fetch(0, 0)
        cur = None
        for j in range(MC):
            m0 = j * 512
            colsum_ps = csps.tile([1, 512], F32, tag="cs", name=f"cs_{j}")
            xr = [xrps.tile([P, 512], F32, tag=f"xr{h}", name=f"xr{h}_{j}")
                  for h in range(2)]
            for i in range(NB):
                first, last = (i == 0), (i == NB - 1)
                if i % 4 == 0:
                    cur = blk
                    if i + 4 < NB:
                        blk = fetch(j, i + 4)
                    elif j + 1 < MC:
                        blk = fetch(j + 1, 0)
                a2 = cur[:, (i % 4) * 512:(i % 4) * 512 + 512]
                for h in range(2):
                    nc.tensor.matmul(
                        xr[h][:],
                        vT[:, i * C + h * P: i * C + h * P + P],
                        a2, start=first, stop=last)
                nc.tensor.matmul(colsum_ps[:], invrs_bf[:, i:i + 1], a2,
                                 start=first, stop=last)

            # tail for this chunk. xr is copied to SBUF immediately so the
            # xr psum banks are free for chunk j+1's accumulation while the
            # rest of the tail (reciprocal chain, t conv, BN, lrelu) runs.
            # invcs broadcast runs on the otherwise-idle GpSimd engine.
            xr_sb = []
            for k in range(2):
                xc = tails.tile([P, 512], F32, tag=f"xc{k}")
                nc.vector.tensor_copy(xc[:], xr[k][:])
                xr_sb.append(xc)
            cs_eps = tails.tile([1, 512], F32, tag="cse")
            nc.vector.tensor_scalar_add(cs_eps[:], colsum_ps[:], 1e-9)
            invr = tails.tile([1, 512], F32, tag="invr")
            nc.vector.reciprocal(invr[:], cs_eps[:])
            invcs_bc = tails.tile([P, 512], F32, tag="ib")
            nc.gpsimd.partition_broadcast(invcs_bc[:], invr[:])
            ys = []
            for k in range(2):
                tmp = tails.tile([P, 512], F32, tag=f"tmp{k}")
                nc.vector.tensor_mul(tmp[:], xr_sb[k][:], invcs_bc[:])
                y_h = tails.tile([P, 512], F32R, tag=f"y{k}")
                nc.vector.tensor_sub(
                    y_h[:], xs[:, k * N + m0: k * N + m0 + 512].bitcast(F32),
                    tmp[:])
                ys.append(y_h)
            for ho in range(2):
                t_ps = tpps.tile([P, 512], F32, tag="tp",
                                 name=f"tps{ho}_{j}")
                for k in range(2):
                    nc.tensor.matmul(
                        t_ps[:],
                        wtT_r[:, k * C + ho * P: k * C + ho * P + P],
                        ys[k][:], start=(k == 0), stop=(k == 1))
                bn = tails.tile([P, 512], F32, tag=f"bn{ho}")
                nc.vector.tensor_scalar(bn[:], t_ps[:], g_t[:, ho:ho + 1],
                                        bp_t[:, ho:ho + 1],
                                        mybir.AluOpType.mult,
                                        mybir.AluOpType.add)
                lr = tails.tile([P, 512], F32, tag=f"lr{ho}")
                nc.vector.scalar_tensor_tensor(lr[:], bn[:], 0.2, bn[:],
                                               mybir.AluOpType.mult,
                                               mybir.AluOpType.max)
                o_t = tails.tile([P, 512], F32, tag=f"o{ho}")
                nc.vector.tensor_add(
                    o_t[:], lr[:],
                    xs[:, ho * N + m0: ho * N + m0 + 512].bitcast(F32))
                nc.sync.dma_start(
                    out_d[ho * P:(ho + 1) * P, m0:m0 + 512], o_t[:])


# ---------------------------------------------------------------------------
# host-side wrapper
# ---------------------------------------------------------------------------
_NC_CACHE = {}


def _get_nc(N=4096):
    if N not in _NC_CACHE:
        _NC_CACHE[N] = build_kernel(N=N)
    return _NC_CACHE[N]


def host_prep(wqk, bqk, wv, bv, wt, bt, bn_gamma, bn_beta, bn_mean, bn_var):
    wqk = np.asarray(wqk, np.float32)
    wv = np.asarray(wv, np.float32)
    wt = np.asarray(wt, np.float32)
    g = (np.asarray(bn_gamma, np.float32)
         / np.sqrt(np.asarray(bn_var, np.float32) + BN_EPS))
    bp = np.asarray(bn_beta, np.float32) - np.asarray(bn_mean, np.float32) * g
    btp = np.asarray(bt, np.float32) - wt @ np.asarray(bv, np.float32)
    bp_eff = btp * g + bp
    return {
        "wqkT": np.ascontiguousarray(wqk.T),
        "wvT": np.ascontiguousarray(wv.T),
        "wtT": np.ascontiguousarray(wt.T),
        "bqk": np.asarray(bqk, np.float32).reshape(CQK, 1),
        "g": g.reshape(C, 1),
        "bp": bp_eff.reshape(C, 1),
    }


def kernel(x, xyz, wqk, bqk, wv, bv, wt, bt, bn_gamma, bn_beta, bn_mean,
           bn_var, _profile=False):
    from concourse.bass_utils import run_bass_kernel_spmd

    x = np.asarray(x, np.float32)
    xyz = np.asarray(xyz, np.float32)
    B, Cc, N = x.shape
    assert Cc == C and B == 8
    nc = _get_nc(N)
    wmap = host_prep(wqk, bqk, wv, bv, wt, bt, bn_gamma, bn_beta, bn_mean,
                     bn_var)
    in_maps = [
        {"x": np.ascontiguousarray(x[b]),
         "xyz": np.ascontiguousarray(xyz[b]), **wmap}
        for b in range(B)
    ]
    res = run_bass_kernel_spmd(nc, in_maps, list(range(8)), trace=_profile)
    out = np.stack([res.results[b]["out"] for b in range(B)], axis=0)
    if _profile:
        return out, res
    return out
